# revision 26
# baseline (speedup 1.0000x reference)
"""Trainium2 Bass kernel for nn_CAM_6949257085456 (v2).

Pure data-parallel over batch: 8 cores x 64 samples, each core split in 2
halves of 512 activation rows so the branch (attention) stage of half 0
overlaps the big-matmul DMA stream of half 1.

Layout/engine plan (from the v1 trace: DMA ring 69% busy at ~184 GB/s,
PE 61% busy mostly cold):
  - X stream: host-packed contiguous [h][g][128, 14, 512] bf16 blocks,
    14 x 1.75 MB DMAs per half on the sync (SP) HWDGE ring only.
  - Weights/constants on the scalar (ACT) HWDGE ring, so they never
    stall the X stream.
  - avf scatter + outputs on the gpsimd (SWDGE) ring.
  - Scalar engine does ONLY tanh (batched [128,1024] per sample pair);
    all other evac/bias/relu/residual work moved to the vector engine.
  - Regressors folded (no nonlinearity) and computed as DVE
    tensor_tensor_reduce against broadcast weight rows; final bias added
    on host.

Host-side algebraic folds (exact in fp32):
  - vis path: X @ W_red.T @ W_enc2.T == X @ (W_enc2 @ W_red).T
  - regressors: feats@Wv1.T@Wv2.T == feats @ (Wv2@Wv1).T
Everything fed to the chip is bf16 (fp32 PSUM accumulation).
"""
import sys

if "/opt/trn_rl_repo" not in sys.path:
    sys.path.insert(0, "/opt/trn_rl_repo")

import numpy as np
import ml_dtypes

import concourse.bacc as bacc
import concourse.bass as bass
import concourse.mybir as mybir
import concourse.tile as tile
from concourse import bass_utils
from concourse.ap import AP

BF16 = mybir.dt.bfloat16
F32 = mybir.dt.float32
AF = mybir.ActivationFunctionType
ALU = mybir.AluOpType

B, T, DA, DV, DH = 512, 16, 512, 25088, 128
NCORES = 8
S = B // NCORES            # samples per core (64)
R = S * T                  # rows per core (1024)
NH = 2                     # halves per core
RH = R // NH               # rows per half (512)
KC = DV // 128             # contraction chunks (196)
WT = 14                    # chunks per weight tile / per X DMA group
NG = KC // WT              # 14 DMA groups per half
SCALE = 1.0 / 16.0         # 1/sqrt(256)

_CACHE = {}


def _build():
    nc = bacc.Bacc("TRN2", target_bir_lowering=False, debug=False)

    xarr = nc.dram_tensor("xarr", [NH, NG, 128, WT, RH], BF16, kind="ExternalInput")
    f1arr = nc.dram_tensor("f1arr", [4, 128, R], BF16, kind="ExternalInput")
    warr = nc.dram_tensor("warr", [WT, 128, WT, DH], BF16, kind="ExternalInput")
    wenc1T = nc.dram_tensor("wenc1T", [DA, DH], BF16, kind="ExternalInput")
    b1 = nc.dram_tensor("b1", [DH, 1], F32, kind="ExternalInput")
    b2 = nc.dram_tensor("b2", [DH, 1], F32, kind="ExternalInput")
    kronA = nc.dram_tensor("kronA", [128, 128], BF16, kind="ExternalInput")
    kronV = nc.dram_tensor("kronV", [128, 128], BF16, kind="ExternalInput")
    waT = nc.dram_tensor("waT", [128, 32], BF16, kind="ExternalInput")
    wcaT = nc.dram_tensor("wcaT", [256, 32], BF16, kind="ExternalInput")
    whq = nc.dram_tensor("whq", [32, 2, 4, 128], BF16, kind="ExternalInput")
    wvrep = nc.dram_tensor("wvrep", [128, 256], BF16, kind="ExternalInput")
    warep = nc.dram_tensor("warep", [128, 256], BF16, kind="ExternalInput")
    ident = nc.dram_tensor("ident", [128, 128], BF16, kind="ExternalInput")

    vouts_raw = nc.dram_tensor("vouts_raw", [NH, 128, 8], F32, kind="ExternalOutput")
    aouts_raw = nc.dram_tensor("aouts_raw", [NH, 128, 8], F32, kind="ExternalOutput")

    from contextlib import ExitStack

    with tile.TileContext(nc) as tc:
        with ExitStack() as stack:
            ec = stack.enter_context
            cpool = ec(tc.tile_pool(name="const", bufs=1))
            wpool = ec(tc.tile_pool(name="wred", bufs=1))
            xpool = ec(tc.tile_pool(name="xin", bufs=4))
            actpool = ec(tc.tile_pool(name="actsb", bufs=4))
            rowpool = ec(tc.tile_pool(name="rows", bufs=4))
            avfpool = ec(tc.tile_pool(name="avf", bufs=6))
            gsbpool = ec(tc.tile_pool(name="gsb", bufs=3))
            attsbpool = ec(tc.tile_pool(name="attsb", bufs=3))
            htsbpool = ec(tc.tile_pool(name="htsb", bufs=4))
            featpool = ec(tc.tile_pool(name="featsb", bufs=3))
            scrpool = ec(tc.tile_pool(name="scr", bufs=2))
            vispool = ec(tc.tile_pool(name="vis_ps", bufs=1, space="PSUM"))
            attpool = ec(tc.tile_pool(name="att_ps", bufs=2, space="PSUM"))
            trpool = ec(tc.tile_pool(name="tr_ps", bufs=1, space="PSUM"))
            gpool = ec(tc.tile_pool(name="g_ps", bufs=1, space="PSUM"))
            htpool = ec(tc.tile_pool(name="ht_ps", bufs=1, space="PSUM"))
            outapool = ec(tc.tile_pool(name="outa_ps", bufs=1, space="PSUM"))
            outvpool = ec(tc.tile_pool(name="outv_ps", bufs=1, space="PSUM"))

            # ---- constants / weights: scalar (ACT) ring ----
            ident_sb = cpool.tile([128, 128], BF16)
            nc.sync.dma_start(ident_sb[:], ident[:])
            wenc1_sb = cpool.tile([128, 4, DH], BF16)
            nc.sync.dma_start(
                wenc1_sb[:], wenc1T.ap().rearrange("(c p) f -> p c f", p=128)
            )
            b1_sb = cpool.tile([DH, 1], F32)
            nc.sync.dma_start(b1_sb[:], b1[:])
            b2_sb = cpool.tile([DH, 1], F32)
            nc.sync.dma_start(b2_sb[:], b2[:])
            f1_sb = cpool.tile([128, 4, R], BF16)
            for c in range(4):
                nc.sync.dma_start(f1_sb[:, c, :], f1arr[c])

            # Interleave weight tiles with half-0 X chunks so the PE starts
            # the vis matmul ~2.5us in instead of after a 20us weight phase.
            w_tiles = []
            xk_h0 = []
            for g in range(WT):
                wt = wpool.tile([128, WT, DH], BF16, name=f"wt{g}")
                nc.sync.dma_start(wt[:], warr[g])
                w_tiles.append(wt)
                xk = xpool.tile([128, WT, RH], BF16, tag="xk")
                nc.sync.dma_start(xk[:], xarr[0, g])
                xk_h0.append(xk)

            kronA_sb = cpool.tile([128, 128], BF16)
            nc.sync.dma_start(kronA_sb[:], kronA[:])
            kronV_sb = cpool.tile([128, 128], BF16)
            nc.sync.dma_start(kronV_sb[:], kronV[:])
            wa_sb = cpool.tile([128, 32], BF16)
            nc.sync.dma_start(wa_sb[:], waT[:])
            wca_sb = cpool.tile([128, 2, 32], BF16)
            nc.sync.dma_start(
                wca_sb[:], wcaT.ap().rearrange("(c p) f -> p c f", p=128)
            )
            whq_sb = cpool.tile([32, 2, 4, 128], BF16)
            nc.sync.dma_start(whq_sb[:], whq[:])
            wvrep_sb = cpool.tile([128, 256], BF16)
            nc.sync.dma_start(wvrep_sb[:], wvrep[:])
            warep_sb = cpool.tile([128, 256], BF16)
            nc.sync.dma_start(warep_sb[:], warep[:])

            # per-core regressor accumulators (one column per avf tile)
            vred_sb = cpool.tile([128, 16], F32)
            ared_sb = cpool.tile([128, 16], F32)

            for h in range(NH):
                rsl = slice(h * RH, (h + 1) * RH)

                # ---- aud encoder: audT[e, r] (runs while first X chunk lands)
                aud_ps = attpool.tile([128, RH], F32, tag="att", name=f"aud{h}")
                for c in range(4):
                    nc.tensor.matmul(
                        aud_ps[:],
                        wenc1_sb[:, c, :],
                        f1_sb[:, c, rsl],
                        start=(c == 0),
                        stop=(c == 3),
                    )
                audT_sb = actpool.tile([128, RH], BF16, tag="act")
                nc.vector.tensor_scalar_add(audT_sb[:], aud_ps[:], b1_sb[:])

                # ---- vis encoder (the big one): visT[e, r] ----
                vis_ps = vispool.tile([128, RH], F32)
                for g in range(NG):
                    if h == 0:
                        xk = xk_h0[g]
                    else:
                        xk = xpool.tile([128, WT, RH], BF16, tag="xk")
                        nc.sync.dma_start(xk[:], xarr[h, g])
                    for j in range(WT):
                        k = g * WT + j
                        nc.tensor.matmul(
                            vis_ps[:],
                            w_tiles[g][:, j, :],
                            xk[:, j, :],
                            start=(k == 0),
                            stop=(k == KC - 1),
                        )
                visT_sb = actpool.tile([128, RH], BF16, tag="act")
                nc.vector.tensor_scalar_add(visT_sb[:], vis_ps[:], b2_sb[:])

                # ---- build quadrant-packed avf tiles: 4 samples @32-row
                # spacing, cols [aud 128 | vis 128].  Scatter DMAs copy 32
                # consecutive rows where possible (rows 16..31 of each
                # quadrant carry duplicated neighbor data, never read as
                # matmul input); they ride the scalar HWDGE ring so the X
                # stream on the sync ring is undisturbed.
                avf_tiles = []
                for jg in range(RH // 128):
                    rows_sb = {}
                    for bi, src in ((0, audT_sb), (1, visT_sb)):
                        tr_ps = trpool.tile([128, 128], BF16, tag="tr",
                                            name=f"tr{h}_{jg}_{bi}")
                        nc.tensor.transpose(
                            tr_ps[:], src[:, jg * 128:(jg + 1) * 128], ident_sb[:]
                        )
                        rs = rowpool.tile([128, 128], BF16, tag=f"rows{bi}",
                                          name=f"rows{h}_{jg}_{bi}")
                        nc.vector.tensor_copy(rs[:], tr_ps[:])
                        rows_sb[bi] = rs
                    for u in range(2):
                        avf_t = avfpool.tile([128, 256], BF16, tag="avf",
                                             name=f"avf{h}_{jg}_{u}")
                        if u == 1:
                            nc.vector.memset(avf_t[96:128, :], 0.0)
                        for bi in (0, 1):
                            rs = rows_sb[bi]
                            for q in range(4):
                                s0 = 64 * u + 16 * q
                                n = 32 if s0 + 32 <= 128 else 16
                                nc.sync.dma_start(
                                    avf_t[32 * q:32 * q + n,
                                          128 * bi:128 * bi + 128],
                                    rs[s0:s0 + n, :],
                                )
                        avf_tiles.append(avf_t)

                # ---- branch stage, 4 samples per avf tile ----
                for a, avf_t in enumerate(avf_tiles):
                    # G = W_aff-kron @ avf  (both branches, 4 samples each)
                    g_ps = gpool.tile([128, 256], F32, tag="g", name=f"g{h}_{a}")
                    nc.tensor.matmul(g_ps[:, 0:128], kronA_sb[:],
                                     avf_t[:, 0:128], start=True, stop=True)
                    nc.tensor.matmul(g_ps[:, 128:256], kronV_sb[:],
                                     avf_t[:, 128:256], start=True, stop=True)
                    g_sb = gsbpool.tile([128, 256], BF16, tag="gsb")
                    nc.vector.tensor_copy(g_sb[:], g_ps[:])

                    out_ps = {
                        0: outapool.tile([128, 128], F32, tag="outa",
                                         name=f"outa{h}_{a}"),
                        1: outvpool.tile([128, 128], F32, tag="outv",
                                         name=f"outv{h}_{a}"),
                    }
                    for q in range(4):
                        att_ps = attpool.tile([128, 512], F32, tag="att",
                                              name=f"att{h}_{a}_{q}")
                        for jh in range(2):
                            nc.tensor.matmul(
                                att_ps[:, 256 * jh:256 * jh + 256],
                                avf_t[32 * q:32 * q + 16,
                                      128 * jh:128 * jh + 128],
                                g_sb[32 * q:32 * q + 16, :],
                                start=True,
                                stop=True,
                                tile_position=(32 * q, 0),
                            )
                        att_sb = attsbpool.tile([128, 512], BF16, tag="attsb")
                        nc.scalar.activation(att_sb[:], att_ps[:], AF.Tanh,
                                             scale=SCALE)
                        ht_ps = htpool.tile([32, 256], F32, tag="ht",
                                            name=f"ht{h}_{a}_{q}")
                        for jh in range(2):
                            nc.tensor.matmul(
                                ht_ps[:],
                                wca_sb[:, jh, :],
                                att_sb[:, 256 * jh:256 * jh + 256],
                                start=(jh == 0),
                                stop=False,
                            )
                        nc.tensor.matmul(
                            ht_ps[:],
                            wa_sb[32 * q:32 * q + 16, :],
                            avf_t[32 * q:32 * q + 16, :],
                            start=False,
                            stop=True,
                            tile_position=(32 * q, 0),
                        )
                        ht_sb = htsbpool.tile([32, 256], BF16, tag="htsb")
                        nc.vector.tensor_relu(ht_sb[:], ht_ps[:])
                        for bi in range(2):
                            nc.tensor.matmul(
                                out_ps[bi][:],
                                whq_sb[:, bi, q, :],
                                ht_sb[:, 128 * bi:128 * bi + 128],
                                start=(q == 0),
                                stop=(q == 3),
                            )

                    # residual + folded regressors (vector engine)
                    feats_sb = featpool.tile([128, 256], BF16, tag="feat")
                    for bi in range(2):
                        nc.vector.tensor_add(
                            feats_sb[:, 128 * bi:128 * bi + 128],
                            out_ps[bi][:],
                            avf_t[:, 128 * bi:128 * bi + 128],
                        )
                    col = h * 8 + a
                    for red_sb, wrep in ((vred_sb, wvrep_sb), (ared_sb, warep_sb)):
                        scr = scrpool.tile([128, 256], F32, tag="scr")
                        nc.vector.tensor_mul(scr[:], feats_sb[:], wrep[:])
                        nc.vector.reduce_sum(
                            red_sb[:, col:col + 1], scr[:],
                            axis=mybir.AxisListType.X,
                        )

                nc.sync.dma_start(vouts_raw[h], vred_sb[:, h * 8:h * 8 + 8])
                nc.sync.dma_start(aouts_raw[h], ared_sb[:, h * 8:h * 8 + 8])

    nc.compile()
    return nc


def _prep_shared(inputs):
    f32 = np.float32
    bf = ml_dtypes.bfloat16
    W_enc1 = np.asarray(inputs["W_enc1"], f32)
    W_enc2 = np.asarray(inputs["W_enc2"], f32)
    W_red = np.asarray(inputs["W_red"], f32)
    W2r = W_enc2 @ W_red                                    # [128, 25088]
    b2v = W_enc2 @ np.asarray(inputs["b_red"], f32) + np.asarray(inputs["b_enc2"], f32)
    wv = (np.asarray(inputs["Wv2"], f32) @ np.asarray(inputs["Wv1"], f32))[0]
    cv = float((np.asarray(inputs["Wv2"], f32) @ np.asarray(inputs["bv1"], f32)
                + np.asarray(inputs["bv2"], f32))[0])
    wa = (np.asarray(inputs["Wa2"], f32) @ np.asarray(inputs["Wa1"], f32))[0]
    ca = float((np.asarray(inputs["Wa2"], f32) @ np.asarray(inputs["ba1"], f32)
                + np.asarray(inputs["ba2"], f32))[0])

    W_affa = np.asarray(inputs["W_affa"], f32)
    W_affv = np.asarray(inputs["W_affv"], f32)
    kA = np.zeros((128, 128), f32)
    kV = np.zeros((128, 128), f32)
    for q in range(4):
        kA[32 * q:32 * q + 16, 32 * q:32 * q + 16] = W_affa.T
        kV[32 * q:32 * q + 16, 32 * q:32 * q + 16] = W_affv.T

    waT = np.zeros((128, 32), f32)
    for q in range(4):
        waT[32 * q:32 * q + 16] = np.asarray(inputs["W_a"], f32).T
    wcaT = np.asarray(inputs["W_ca"], f32).T                # [256, 32]

    whq = np.zeros((32, 2, 4, 128), f32)
    for q in range(4):
        whq[:, 0, q, 32 * q:32 * q + 16] = np.asarray(inputs["W_ha"], f32).T
        whq[:, 1, q, 32 * q:32 * q + 16] = np.asarray(inputs["W_hv"], f32).T

    # weight tiles: warr[g, p, j, f] = W2r.T[(g*14+j)*128+p, f]
    warr = np.ascontiguousarray(
        W2r.T.reshape(WT, WT, 128, DH).transpose(0, 2, 1, 3)
    ).astype(bf)

    shared = {
        "warr": warr,
        "wenc1T": np.ascontiguousarray(W_enc1.T).astype(bf),
        "b1": np.asarray(inputs["b_enc1"], f32).reshape(128, 1),
        "b2": b2v.reshape(128, 1),
        "kronA": kA.astype(bf),
        "kronV": kV.astype(bf),
        "waT": waT.astype(bf),
        "wcaT": wcaT.astype(bf),
        "whq": whq.astype(bf),
        "wvrep": np.ascontiguousarray(np.broadcast_to(wv, (128, 256))).astype(bf),
        "warep": np.ascontiguousarray(np.broadcast_to(wa, (128, 256))).astype(bf),
        "ident": np.eye(128, dtype=f32).astype(bf),
    }
    return shared, cv, ca


def kernel(**inputs):
    if "nc" not in _CACHE:
        _CACHE["nc"] = _build()
    nc = _CACHE["nc"]

    bf = ml_dtypes.bfloat16
    shared, cv, ca = _prep_shared(inputs)

    f1 = np.asarray(inputs["f1_norm"], np.float32).reshape(B * T, DA)
    f2 = np.asarray(inputs["f2_norm"], np.float32).reshape(B * T, DV)

    in_maps = []
    for c in range(NCORES):
        rs = slice(c * R, (c + 1) * R)
        m = dict(shared)
        # xarr[h, g, p, j, r] = X.T[(g*14+j)*128+p, h*512+r]
        f2c = f2[rs]                                        # [1024, 25088]
        m["xarr"] = np.ascontiguousarray(
            f2c.reshape(NH, RH, NG, WT, 128).transpose(0, 2, 4, 3, 1)
        ).astype(bf)
        f1c = f1[rs]                                        # [1024, 512]
        m["f1arr"] = np.ascontiguousarray(
            f1c.reshape(R, 4, 128).transpose(1, 2, 0)
        ).astype(bf)
        in_maps.append(m)

    import os

    res = bass_utils.run_bass_kernel_spmd(
        nc,
        in_maps,
        core_ids=list(range(NCORES)),
        trace=bool(os.environ.get("KERNEL_TRACE")),
    )
    _CACHE["last_results"] = res

    def gather(key, const):
        outs = []
        for r in res.results:
            raw = r[key].reshape(NH, 4, 2, 16, 8)           # h, q, s, t, a
            v = raw[:, :, 0, :, :]                          # [h, q, t, a]
            v = v.transpose(0, 3, 1, 2).reshape(S, T)       # h, a, q -> samples
            outs.append(v)
        return (np.concatenate(outs, axis=0) + const).astype(np.float32)

    vouts = gather("vouts_raw", cv)
    aouts = gather("aouts_raw", ca)
    return vouts, aouts
